# revision 25
# baseline (speedup 1.0000x reference)
"""Trainium2 Bass kernel for the DAMIC-style model:
embedding lookup -> 3x Conv1d(+ReLU+max-pool over tokens) -> BiLSTM over T -> sigmoid head.

Sharding: data-parallel over batch (B=32 -> 4 per core on 8 cores); weights
replicated; both LSTM directions computed per core on its own batch shard; the
host only reshapes/concats.

Conv data path: embeddings gathered token-major [tok, emb]; all three 128-col
emb slabs are transposed on the PE into one PSUM bank per gather chunk; dims
0:256 are cast to fp8 planes consumed by DoubleRow fp8 matmuls (2 K-subtiles
of 128 per MM); dims 256:300 (+pad) are copied bf16 into the pair/single tap
packing tile. No XBAR DMA transposes - the two HWDGE queues carry only weights
and the per-chunk g2 shift.

LSTM: fwd/rev run as independent chains; steps are pumped one at a time
between conv psum-tile groups so the recurrence latency hides under conv PE
work. Gate order (i,f,o,g) with g-gate weights doubled host-side so a single
sigmoid yields sigma(2g) = (tanh(g)+1)/2. All gate pre-activations are scaled
x16 host-side (wih/whh/bias) so whh survives the fp8 cast; the sigmoid applies
scale=1/16. The gate bias rides the kk=0 input-projection matmul via a
constant-1 feats row. whh and the hidden states are fp8 (FWL weight loads);
i*tanh(g) is one fused affine_mul_reduce DVE op. Head weights fp8 x32, head
sigmoid scale=1/32.
"""

import sys

sys.path.insert(0, "/opt/trn_rl_repo")

import numpy as np
import ml_dtypes

BF16 = ml_dtypes.bfloat16
F8 = ml_dtypes.float8_e4m3

VOCAB, EMB = 32000, 300
EMB_PAD = 384
NF = 100
NFP = 112                     # fp8 DR weight tile padded filter stride (16B mult)
FSIZES = (3, 4, 5)
NTAPS = 12
HID, OUT = 256, 32
B, T, L = 32, 64, 48
NCORES = 8
B_CORE = B // NCORES          # 4
S_CORE = B_CORE * T           # 256 sentences/core, ordered (t, b)
NTOK = S_CORE * L             # 12288
CH_T = 8                      # timesteps per conv chunk
NCH = T // CH_T               # 8 conv chunks
CHSENT = CH_T * B_CORE        # 32 sentences per chunk
CHTOK = CHSENT * L            # 1536 tokens per chunk
GCH = CHTOK // 128            # 12 gather chunks (128 tokens) per conv chunk
SENT_PER_PS = 8               # sentences per conv psum tile (= 2 timesteps)
PS_COLS = SENT_PER_PS * L     # 384
G4 = 4 * HID
WS = 16.0                     # gate pre-activation scale (fp8 whh headroom)
HS = 32.0                     # head weight scale
CONV_ORDER = [0, 7, 1, 6, 2, 5, 3, 4]
PAIRS_OF = {3: [(0, 1)], 4: [(0, 1), (2, 3)], 5: [(0, 1), (2, 3)]}
SINGLES_OF = {3: [2], 4: [], 5: [4]}
PAIR_LIST = [(3, 0), (4, 0), (4, 2), (5, 0), (5, 2)]
PAIR_IDX = {p: i for i, p in enumerate(PAIR_LIST)}
SINGLE_LIST = [(3, 2), (5, 4)]
SINGLE_IDX = {p: i for i, p in enumerate(SINGLE_LIST)}

_PROG = None


def build_program():
    import concourse.bass as bass
    import concourse.tile as tile
    from concourse import bacc, mybir

    f32 = mybir.dt.float32
    bf16 = mybir.dt.bfloat16
    f8 = mybir.dt.float8e4
    AF = mybir.ActivationFunctionType
    ALU = mybir.AluOpType

    nc = bacc.Bacc("TRN2", target_bir_lowering=False, debug=False)

    idx_d = nc.dram_tensor("idx_w", [128, NTOK // 128], mybir.dt.int32, kind="ExternalInput").ap()
    iden_d = nc.dram_tensor("iden", [128, 128], bf16, kind="ExternalInput").ap()
    iden8_d = nc.dram_tensor("iden8", [128, 128], f8, kind="ExternalInput").ap()
    # packed embedding row: 256B fp8 (dims 0:256) + 88B bf16 (dims 256:300) + pad
    emb_d = nc.dram_tensor("emb_p", [VOCAB, EMB_PAD], mybir.dt.uint8, kind="ExternalInput").ap()
    convw8_d = nc.dram_tensor("convw8", [128, NTAPS, 2, NFP], f8, kind="ExternalInput").ap()
    convw2_d = nc.dram_tensor("convw2", [128, 5 * NF], bf16, kind="ExternalInput").ap()
    convwS_d = nc.dram_tensor("convwS", [128, 2 * NF], bf16, kind="ExternalInput").ap()
    convb_d = nc.dram_tensor("convb", [NF, 3], f32, kind="ExternalInput").ap()
    # wih rows 0:100 = weights.T (x16, g-gate x2); row 100 of kk=0 = bias
    wih_d = nc.dram_tensor("wih", [NF + 1, 3, 2, G4], bf16, kind="ExternalInput").ap()
    whh_d = nc.dram_tensor("whh", [128, 2, 2, 8, 128], f8, kind="ExternalInput").ap()
    headw_d = nc.dram_tensor("headw", [128, 4, OUT], f8, kind="ExternalInput").ap()
    headb_d = nc.dram_tensor("headb", [OUT, 1], f32, kind="ExternalInput").ap()
    out_d = nc.dram_tensor("out_t", [OUT, S_CORE], f32, kind="ExternalOutput").ap()

    tap_of = {3: 0, 4: 3, 5: 7}

    with tile.TileContext(nc) as tc:
        with (
            tc.tile_pool(name="const", bufs=1) as const,
            tc.tile_pool(name="gatA", bufs=3) as gatA,
            tc.tile_pool(name="gatB", bufs=3) as gatB,
            tc.tile_pool(name="gtok", bufs=48) as gtok,
            tc.tile_pool(name="small", bufs=3) as small,
            tc.tile_pool(name="cstate", bufs=2) as cstate,
            tc.tile_pool(name="cpsum", bufs=1, space="PSUM") as cpsum,
            tc.tile_pool(name="tpsum", bufs=2, space="PSUM") as tpsum,
            tc.tile_pool(name="xpsum", bufs=1, space="PSUM") as xpsum,
            tc.tile_pool(name="spsum", bufs=1, space="PSUM") as spsum,
            tc.tile_pool(name="hpsum", bufs=1, space="PSUM") as hpsum,
        ):
            # gather-critical loads first (gpsimd queue feeds the gathers)
            idx_sb = const.tile([128, NTOK // 128], mybir.dt.int32)
            nc.gpsimd.dma_start(out=idx_sb[:], in_=idx_d[:])
            iden_sb = const.tile([128, 128], bf16)
            nc.gpsimd.dma_start(out=iden_sb[:], in_=iden_d[:])
            iden8_sb = const.tile([128, 128], f8)
            nc.gpsimd.dma_start(out=iden8_sb[:], in_=iden8_d[:])
            # small conv weights on the sync HWDGE queue (ahead of the
            # per-chunk g2 shift DMAs, so those never sit behind big loads);
            # the big LSTM/head weights ride the scalar HWDGE queue, whose
            # engine is idle until the first relu
            convw8_sb = const.tile([128, NTAPS, 2, NFP], f8)
            nc.sync.dma_start(
                out=convw8_sb[:].rearrange("p a b c -> p (a b c)"),
                in_=convw8_d[:].rearrange("p a b c -> p (a b c)"),
            )
            convw2_sb = const.tile([128, 5 * NF], bf16)
            nc.sync.dma_start(out=convw2_sb[:], in_=convw2_d[:])
            convwS_sb = const.tile([128, 2 * NF], bf16)
            nc.sync.dma_start(out=convwS_sb[:], in_=convwS_d[:])
            convb_sb = const.tile([NF, 3], f32)
            nc.sync.dma_start(out=convb_sb[:], in_=convb_d[:])
            wih_sb = const.tile([NF + 1, 3, 2, G4], bf16)
            nc.scalar.dma_start(out=wih_sb[:], in_=wih_d[:])
            whh_sb = const.tile([128, 2, 2, 8, 128], f8)
            nc.scalar.dma_start(out=whh_sb[:], in_=whh_d[:])
            headw_sb = const.tile([128, 4, OUT], f8)
            nc.scalar.dma_start(out=headw_sb[:], in_=headw_d[:])
            headb_sb = const.tile([OUT, 1], f32)
            nc.scalar.dma_start(out=headb_sb[:], in_=headb_d[:])

            feats = [const.tile([128, S_CORE], bf16, tag=f"f{fs}", name=f"f{fs}") for fs in FSIZES]
            for fi in range(3):
                # constant-1 row 100 folds the (x16-scaled) gate bias into the
                # kk=0 xg matmul (wih row 100). Partition writes must start at
                # a 32-aligned base, so set rows 96:128; the conv reduce_max
                # overwrites rows 96:100 before anything reads them.
                nc.vector.memset(feats[fi][96:128, :], 1.0)
            # xgT: [128, (d, t, gt, b)] bf16 = x16-scaled input projections
            xgT = const.tile([128, 2, T, 8, B_CORE], bf16)
            # hseq: [128, slot, dir, ktile, b] fp8; slot 0 = h0 = 0
            hseq = const.tile([128, T + 1, 2, 2, B_CORE], f8)
            nc.vector.memset(hseq[:, 0], 0.0)
            c_prev = []
            for d in range(2):
                c0 = cstate.tile([128, 2 * B_CORE], f32, tag=f"c{d}", name=f"c0{d}")
                nc.vector.memset(c0[:], 0.0)
                c_prev.append(c0)
            # AMR per-partition accumulator scratch (result unused)
            amr_scratch = [
                const.tile([128, 1], f32, tag=f"amr{d}", name=f"amr{d}") for d in range(2)
            ]

            # ---------------- LSTM dir-step (independent chains) ----------------
            def lstm_dir_step(d, s):
                tt = s if d == 0 else T - 1 - s
                rslot = (s if d == 0 else (T + 1 - s)) if s > 0 else 0
                ps = spsum.tile([128, 32], f32, tag=f"gp{d}", name=f"gp{d}")
                # 16*whh @ h (fp8, FWL weight loads)
                for gt in range(8):
                    for kk in range(2):
                        nc.tensor.matmul(
                            ps[:, 4 * gt : 4 * gt + 4],
                            whh_sb[:, d, kk, gt, :],
                            hseq[:, rslot, d, kk, :],
                            start=(kk == 0),
                            stop=(kk == 1),
                        )
                gates = small.tile([128, 32], f32, tag=f"gates{d}", name=f"gates{d}")
                nc.vector.tensor_add(gates[:], ps[:], xgT[:, d, tt].rearrange("p g b -> p (g b)"))
                # cols 0:8=i, 8:16=f, 16:24=o, 24:32=sigma(2g); scale undoes x16
                sig = small.tile([128, 32], f32, tag=f"sig{d}", name=f"sig{d}")
                nc.scalar.activation(sig[:], gates[:], AF.Sigmoid, scale=1.0 / WS)
                # u = sig_i * (2*sig_2g - 1) = sig_i * tanh(g), one fused DVE op
                u = small.tile([128, 8], f32, tag=f"u{d}", name=f"u{d}")
                nc.vector.affine_mul_reduce(
                    u[:], amr_scratch[d][:], sig[:, 24:32], sig[:, 0:8], 2.0, -1.0
                )
                a = small.tile([128, 8], f32, tag=f"a{d}", name=f"a{d}")
                nc.vector.tensor_mul(a[:], sig[:, 8:16], c_prev[d][:])
                cn = cstate.tile([128, 2 * B_CORE], f32, tag=f"c{d}", name=f"c{d}")
                nc.vector.tensor_add(cn[:], a[:], u[:])
                c_prev[d] = cn
                thc = small.tile([128, 8], f32, tag=f"thc{d}", name=f"thc{d}")
                nc.scalar.activation(thc[:], cn[:], AF.Tanh)
                nc.vector.tensor_mul(hseq[:, tt + 1, d], sig[:, 16:24], thc[:])

            prog = {"fwd": 0, "rev": 0}
            done_chunks = set()

            def ready(dname):
                k = prog[dname]
                if k >= T:
                    return False
                ch = (k // CH_T) if dname == "fwd" else ((T - 1 - k) // CH_T)
                return ch in done_chunks

            def pump(budget):
                n = 0
                while n < budget:
                    progress = False
                    for dname, d in (("fwd", 0), ("rev", 1)):
                        if n >= budget:
                            break
                        if ready(dname):
                            lstm_dir_step(d, prog[dname])
                            prog[dname] += 1
                            n += 1
                            progress = True
                    if not progress:
                        break

            # ---------------- conv chunk ----------------
            def conv_chunk(sc):
                gA = gatA.tile([128, 2, CHTOK], f8, tag="gA", name="gA")
                g2 = gatB.tile([128, CHTOK], bf16, tag="g2", name="g2")
                for c in range(GCH):
                    gc = GCH * sc + c
                    gt = gtok.tile([128, EMB_PAD], mybir.dt.uint8, tag="gt", name="gt")
                    nc.gpsimd.indirect_dma_start(
                        out=gt[:], out_offset=None, in_=emb_d[:],
                        in_offset=bass.IndirectOffsetOnAxis(
                            ap=idx_sb[:, gc : gc + 1], axis=0
                        ),
                    )
                    # all three 128-col slabs via PE transpose into one PSUM
                    # bank (two fp8 planes + the bf16 tail whose rows 44:64
                    # land zero from the pad bytes); fp8 transpose mode writes
                    # with element step 2, hence the strided views. Rows
                    # 64:128 of g2 come from the shift DMA below.
                    tp = tpsum.tile([128, 384], bf16, tag="tp", name="tp")
                    for e in range(2):
                        nc.tensor.transpose(
                            tp[:, 128 * e : 128 * (e + 1)]
                            .bitcast(f8)
                            .rearrange("p (c two) -> p c two", two=2)[:, :, 0:1],
                            gt[:, 128 * e : 128 * (e + 1)].bitcast(f8),
                            iden8_sb[:],
                        )
                    nc.tensor.transpose(
                        tp[0:64, 256:384], gt[:, 256:384].bitcast(bf16), iden_sb[:]
                    )
                    nc.vector.tensor_copy(
                        out=gA[:, :, 128 * c : 128 * (c + 1)],
                        in_=tp[:, 0:256]
                        .bitcast(f8)
                        .rearrange("p (e c two) -> p e c two", e=2, two=2)[:, :, :, 0:1],
                    )
                    nc.vector.tensor_copy(
                        out=g2[0:64, 128 * c : 128 * (c + 1)],
                        in_=tp[0:64, 256:384],
                    )
                # tap-tail pairing: rows 64:128 of g2 = rows 0:64 shifted by 1
                # token (rows 44:64 are zero -> rows 108:128 stay finite zeros)
                nc.sync.dma_start(
                    out=g2[64:128, 0 : CHTOK - 1], in_=g2[0:64, 1:CHTOK]
                )
                nc.vector.memset(g2[64:128, CHTOK - 1 : CHTOK], 0.0)
                for j in range(CHSENT // SENT_PER_PS):  # 4 psum tiles
                    base = PS_COLS * j
                    s0 = CHSENT * sc + SENT_PER_PS * j
                    for fi, fs in enumerate(FSIZES):
                        pt = (3 * j + fi) % 2  # alternate 2 banks for overlap
                        ps = cpsum.tile([NF, PS_COLS], f32, tag=f"ps{pt}", name=f"ps{pt}")
                        mms = []
                        # fp8 DoubleRow MMs over emb dims 0:256, one per tap
                        for k in range(fs):
                            ti = tap_of[fs] + k
                            mms.append(("dr", ti, k, 0))
                        for (pa, pb) in PAIRS_OF[fs]:
                            pi = PAIR_IDX[(fs, pa)]
                            mms.append(("pair", pi, pa, 1))
                        for k in SINGLES_OF[fs]:
                            si = SINGLE_IDX[(fs, k)]
                            mms.append(("single", si, k, 0))
                        for mm, (kind, wi, k, is_pair) in enumerate(mms):
                            n = min(PS_COLS, CHTOK - is_pair - base - k)
                            if kind == "dr":
                                nc.tensor.matmul(
                                    ps[:, 0:n],
                                    convw8_sb[:, wi, :, 0:NF],
                                    gA[:, :, base + k : base + k + n],
                                    start=(mm == 0),
                                    stop=(mm == len(mms) - 1),
                                    perf_mode=mybir.MatmulPerfMode.DoubleRow,
                                )
                            else:
                                wsb = convw2_sb if kind == "pair" else convwS_sb
                                nc.tensor.matmul(
                                    ps[:, 0:n],
                                    wsb[:, wi * NF : (wi + 1) * NF],
                                    g2[:, base + k : base + k + n],
                                    start=(mm == 0),
                                    stop=(mm == len(mms) - 1),
                                )
                        ps3 = ps[:].rearrange("p (s l) -> p s l", l=L)
                        nc.vector.reduce_max(
                            out=feats[fi][0:NF, s0 : s0 + SENT_PER_PS],
                            in_=ps3[:, :, 0 : L - fs + 1],
                            axis=mybir.AxisListType.X,
                        )
                    pump(1)
                for fi in range(3):
                    sl = slice(CHSENT * sc, CHSENT * (sc + 1))
                    nc.scalar.activation(
                        out=feats[fi][0:NF, sl], in_=feats[fi][0:NF, sl], func=AF.Relu,
                        bias=convb_sb[:, fi : fi + 1],
                    )

            def xg_chunk(sc):
                cols = slice(CHSENT * sc, CHSENT * (sc + 1))  # feats cols (t,b)
                for d in range(2):
                    ps = xpsum.tile([128, 8, CHSENT], f32, tag="xp", name="xp")
                    for gt in range(8):
                        for kk in range(3):
                            nc.tensor.matmul(
                                ps[:, gt],
                                wih_sb[:, kk, d, 128 * gt : 128 * (gt + 1)],
                                feats[kk][0 : NF + 1, cols],
                                start=(kk == 0),
                                stop=(kk == 2),
                            )
                    # psum (gt, t, b) -> xgT (t, gt, b), one strided bulk copy
                    nc.scalar.copy(
                        out=xgT[:, d, CH_T * sc : CH_T * (sc + 1), :, :],
                        in_=ps[:].rearrange("p g (t b) -> p t g b", b=B_CORE),
                    )
                    pump(1)

            for sc in CONV_ORDER:
                conv_chunk(sc)
                xg_chunk(sc)
                done_chunks.add(sc)
                pump(2)

            # tail drain: between chain steps, keep the PE fed with dummy
            # matmuls so the HAM clock gate stays at 8/8 (an idle-looking PE
            # is throttled to 1.2 GHz, which doubles every gate matmul). Each
            # filler reads the h history written so far, which pins it into
            # the tail — a dependency-free matmul would get scheduled early.
            def filler(n):
                for dname, d in (("fwd", 0), ("rev", 1)):
                    s = prog[dname]
                    if s < 2:
                        continue
                    # slots surely written and not in flight: fwd [1..s-1],
                    # rev [T-s+2..T]
                    k = min(s - 1, 56)
                    lo = (s - k) if d == 0 else (T - s + 2)
                    mv = hseq[:, lo : lo + k, d]
                    for _ in range(n):
                        fp = hpsum.tile([128, 8 * k], f32, tag="fill", name="fill")
                        nc.tensor.matmul(fp[:], iden8_sb[:], mv)

            while prog["fwd"] < T or prog["rev"] < T:
                before = prog["fwd"] + prog["rev"]
                pump(1)
                if prog["fwd"] + prog["rev"] == before:
                    break
                filler(2)
            assert prog["fwd"] == T and prog["rev"] == T

            # head: out.T[o, (b,t)] = sigmoid((headw*HS) @ h2 / HS + b)
            hp = hpsum.tile([OUT, S_CORE], f32, tag="fill", name="hp")
            for qd in range(4):
                d, kk = qd // 2, qd % 2
                rhs = hseq[:, 1 : T + 1, d, kk, :].rearrange("p t b -> p b t")
                nc.tensor.matmul(
                    hp[:], headw_sb[:, qd, :], rhs, start=(qd == 0), stop=(qd == 3)
                )
            out_sb = small.tile([OUT, S_CORE], f32, tag="outsb", name="outsb")
            nc.scalar.activation(
                out_sb[:], hp[:], AF.Sigmoid, bias=headb_sb[:], scale=1.0 / HS
            )
            nc.gpsimd.dma_start(out=out_d[:], in_=out_sb[:])

    nc.compile()
    return nc


def get_program():
    global _PROG
    if _PROG is None:
        _PROG = build_program()
    return _PROG


# ------------- host-side data prep (reshape/transpose/pad/cast only) -------------

def prep_shared(inputs):
    # packed embedding row: 256 fp8 bytes + 44 bf16 (88 bytes) + 40 zero pad
    emb = np.asarray(inputs["emb"], np.float32)
    emb_p = np.zeros((VOCAB, EMB_PAD), np.uint8)
    emb_p[:, 0:256] = emb[:, 0:256].astype(F8).view(np.uint8)
    emb_p[:, 256:344] = np.ascontiguousarray(emb[:, 256:300].astype(BF16)).view(np.uint8)

    # fp8 DoubleRow weights for emb dims 0:256: [p, tap, j, f] = W[f, 128j+p, k]
    convw8 = np.zeros((128, NTAPS, 2, NFP), np.float32)
    for fs in FSIZES:
        w = np.asarray(inputs[f"conv_w{fs}"], np.float32)  # [NF, EMB, fs]
        for k in range(fs):
            ti = tap_of_host(fs) + k
            blk = w[:, 0:256, k].T.reshape(2, 128, NF).transpose(1, 0, 2)
            convw8[:, ti, :, 0:NF] = blk
    convw8 = convw8.astype(F8)

    convb = np.stack(
        [np.asarray(inputs[f"conv_b{fs}"], np.float32) for fs in FSIZES], axis=1
    )

    # bf16 pair weights for emb dims 256:300 (rows 0:44 tap a, 64:108 tap a+1)
    convw2 = np.zeros((128, 5 * NF), np.float32)
    for i, (fs, ka) in enumerate(PAIR_LIST):
        w = np.asarray(inputs[f"conv_w{fs}"], np.float32)
        convw2[0:44, i * NF : (i + 1) * NF] = w[:, 256:300, ka].T
        convw2[64:108, i * NF : (i + 1) * NF] = w[:, 256:300, ka + 1].T

    # bf16 single-tap weights for emb dims 256:300 (rows 0:44)
    convwS = np.zeros((128, 2 * NF), np.float32)
    for i, (fs, k) in enumerate(SINGLE_LIST):
        w = np.asarray(inputs[f"conv_w{fs}"], np.float32)
        convwS[0:44, i * NF : (i + 1) * NF] = w[:, 256:300, k].T

    perm = np.concatenate(
        [np.arange(0, 256), np.arange(256, 512), np.arange(768, 1024), np.arange(512, 768)]
    )  # i,f,g,o -> i,f,o,g

    wih_h = np.zeros((NF + 1, 3, 2, G4), np.float32)
    whh_h = np.zeros((128, 2, 2, 8, 128), np.float32)
    for d, tag in ((0, "f"), (1, "r")):
        wih = np.asarray(inputs[f"w_ih_{tag}"], np.float32)[perm]
        whh = np.asarray(inputs[f"w_hh_{tag}"], np.float32)[perm]
        bsum = (
            np.asarray(inputs[f"b_ih_{tag}"], np.float32)
            + np.asarray(inputs[f"b_hh_{tag}"], np.float32)
        )[perm].copy()
        # double the g-gate rows (768:1024 after perm) so sigmoid gives sigma(2g)
        wih[768:1024] *= 2.0
        whh[768:1024] *= 2.0
        bsum[768:1024] *= 2.0
        # x16 so whh lands in fp8's normal range; sigmoid applies 1/16
        wih *= WS
        whh *= WS
        bsum *= WS
        for kk in range(3):
            wih_h[0:NF, kk, d, :] = wih[:, kk * NF : (kk + 1) * NF].T
        wih_h[NF, 0, d, :] = bsum  # bias row rides the kk=0 matmul
        whh_h[:, d] = whh.reshape(8, 128, 2, 128).transpose(3, 2, 0, 1)

    headw = np.asarray(inputs["head_w"], np.float32) * HS
    headw_h = headw.T.reshape(4, 128, OUT).transpose(1, 0, 2).astype(F8)
    headb_h = np.asarray(inputs["head_b"], np.float32).reshape(OUT, 1)

    return {
        "emb_p": emb_p,
        "convw8": np.ascontiguousarray(convw8),
        "convw2": np.ascontiguousarray(convw2.astype(BF16)),
        "convwS": np.ascontiguousarray(convwS.astype(BF16)),
        "convb": np.ascontiguousarray(convb),
        "wih": wih_h.astype(BF16),
        "whh": np.ascontiguousarray(whh_h.astype(F8)),
        "headw": np.ascontiguousarray(headw_h),
        "headb": headb_h,
        "iden": np.eye(128, dtype=BF16),
        "iden8": np.eye(128, dtype=F8),
    }


def tap_of_host(fs):
    return {3: 0, 4: 3, 5: 7}[fs]


def prep_core_idx(dialogue, core):
    """(t, b)-ordered token stream; token c*128+p at [p, c]."""
    dia = np.asarray(dialogue[B_CORE * core : B_CORE * (core + 1)], np.int32)
    ids = dia.transpose(1, 0, 2).reshape(-1)  # (t, b, l)
    return np.ascontiguousarray(ids.reshape(NTOK // 128, 128).T)


def kernel(**inputs):
    from concourse.bass_utils import run_bass_kernel_spmd

    nc = get_program()
    shared = prep_shared(inputs)
    dialogue = np.asarray(inputs["dialogue"])
    in_maps = []
    for core in range(NCORES):
        m = dict(shared)
        m["idx_w"] = prep_core_idx(dialogue, core)
        in_maps.append(m)
    res = run_bass_kernel_spmd(nc, in_maps, list(range(NCORES)))
    out = np.zeros((B, T, OUT), np.float32)
    for core in range(NCORES):
        o = res.results[core]["out_t"]  # [32, 256] col = b*64 + t
        out[B_CORE * core : B_CORE * (core + 1)] = o.reshape(OUT, B_CORE, T).transpose(
            1, 2, 0
        )
    return out


# revision 35
# speedup vs baseline: 1.2195x; 1.2195x over previous
"""Trainium2 Bass kernel for the DAMIC-style model:
embedding lookup -> 3x Conv1d(+ReLU+max-pool over tokens) -> BiLSTM over T -> sigmoid head.

Sharding: data-parallel over batch (B=32 -> 4 per core on 8 cores); weights
replicated; both LSTM directions computed per core on its own batch shard; the
host only reshapes/concats.

Conv data path: embeddings gathered token-major [tok, emb]; all three 128-col
emb slabs are transposed on the PE into one PSUM bank per gather chunk; dims
0:256 are cast to fp8 planes consumed by DoubleRow fp8 matmuls (2 K-subtiles
of 128 per MM); dims 256:300 (+pad) are copied bf16 into the pair/single tap
packing tile. No XBAR DMA transposes - the two HWDGE queues carry only weights
and the per-chunk g2 shift.

LSTM: fwd/rev run as independent chains; steps are pumped one at a time
between conv psum-tile groups so the recurrence latency hides under conv PE
work. Gate order (i,f,o,g) with g-gate weights doubled host-side so a single
sigmoid yields sigma(2g) = (tanh(g)+1)/2. All gate pre-activations are scaled
x16 host-side (wih/whh/bias) so whh survives the fp8 cast; the sigmoid applies
scale=1/16. The gate bias rides the kk=0 input-projection matmul via a
constant-1 feats row. whh and the hidden states are fp8 (FWL weight loads);
i*tanh(g) is one fused affine_mul_reduce DVE op. Head weights fp8 x32, head
sigmoid scale=1/32.
"""

import sys

sys.path.insert(0, "/opt/trn_rl_repo")

import numpy as np
import ml_dtypes

BF16 = ml_dtypes.bfloat16
F8 = ml_dtypes.float8_e4m3

VOCAB, EMB = 32000, 300
EMB_PAD = 384
NF = 100
NFP = 112                     # fp8 DR weight tile padded filter stride (16B mult)
FSIZES = (3, 4, 5)
NTAPS = 12
HID, OUT = 256, 32
B, T, L = 32, 64, 48
NCORES = 8
B_CORE = B // NCORES          # 4
S_CORE = B_CORE * T           # 256 sentences/core, ordered (t, b)
NTOK = S_CORE * L             # 12288
CH_T = 8                      # timesteps per conv chunk
NCH = T // CH_T               # 8 conv chunks
CHSENT = CH_T * B_CORE        # 32 sentences per chunk
CHTOK = CHSENT * L            # 1536 tokens per chunk
GCH = CHTOK // 128            # 12 gather chunks (128 tokens) per conv chunk
SENT_PER_PS = 8               # sentences per conv psum tile (= 2 timesteps)
PS_COLS = SENT_PER_PS * L     # 384
G4 = 4 * HID
WS = 16.0                     # gate pre-activation scale (fp8 whh headroom)
HS = 32.0                     # head weight scale
CONV_ORDER = [0, 7, 1, 6, 2, 5, 3, 4]
PAIRS_OF = {3: [(0, 1)], 4: [(0, 1), (2, 3)], 5: [(0, 1), (2, 3)]}
SINGLES_OF = {3: [2], 4: [], 5: [4]}
PAIR_LIST = [(3, 0), (4, 0), (4, 2), (5, 0), (5, 2)]
PAIR_IDX = {p: i for i, p in enumerate(PAIR_LIST)}
SINGLE_LIST = [(3, 2), (5, 4)]
SINGLE_IDX = {p: i for i, p in enumerate(SINGLE_LIST)}

_PROG = None


def build_program():
    import concourse.bass as bass
    import concourse.tile as tile
    from concourse import bacc, mybir

    f32 = mybir.dt.float32
    bf16 = mybir.dt.bfloat16
    f8 = mybir.dt.float8e4
    AF = mybir.ActivationFunctionType
    ALU = mybir.AluOpType

    nc = bacc.Bacc("TRN2", target_bir_lowering=False, debug=False)

    idx_d = nc.dram_tensor("idx_w", [128, NTOK // 128], mybir.dt.int32, kind="ExternalInput").ap()
    iden_d = nc.dram_tensor("iden", [128, 128], bf16, kind="ExternalInput").ap()
    emb_d = nc.dram_tensor("emb_p", [VOCAB, EMB_PAD], bf16, kind="ExternalInput").ap()
    convw8_d = nc.dram_tensor("convw8", [128, NTAPS, 2, NFP], f8, kind="ExternalInput").ap()
    convw2_d = nc.dram_tensor("convw2", [128, 5 * NF], bf16, kind="ExternalInput").ap()
    convwS_d = nc.dram_tensor("convwS", [128, 2 * NF], bf16, kind="ExternalInput").ap()
    convb_d = nc.dram_tensor("convb", [NF, 3], f32, kind="ExternalInput").ap()
    # wih rows 0:100 = weights.T (x16, g-gate x2); row 100 of kk=0 = bias
    wih_d = nc.dram_tensor("wih", [NF + 1, 3, 2, G4], bf16, kind="ExternalInput").ap()
    whh_d = nc.dram_tensor("whh", [128, 2, 2, 8, 128], f8, kind="ExternalInput").ap()
    headw_d = nc.dram_tensor("headw", [128, 4, OUT], f8, kind="ExternalInput").ap()
    headb_d = nc.dram_tensor("headb", [OUT, 1], f32, kind="ExternalInput").ap()
    out_d = nc.dram_tensor("out_t", [OUT, S_CORE], f32, kind="ExternalOutput").ap()

    tap_of = {3: 0, 4: 3, 5: 7}

    with tile.TileContext(nc) as tc:
        with (
            tc.tile_pool(name="const", bufs=1) as const,
            tc.tile_pool(name="gatA", bufs=3) as gatA,
            tc.tile_pool(name="gatB", bufs=3) as gatB,
            tc.tile_pool(name="gtok", bufs=48) as gtok,
            tc.tile_pool(name="small", bufs=3) as small,
            tc.tile_pool(name="cstate", bufs=2) as cstate,
            tc.tile_pool(name="cpsum", bufs=1, space="PSUM") as cpsum,
            tc.tile_pool(name="tpsum", bufs=2, space="PSUM") as tpsum,
            tc.tile_pool(name="xpsum", bufs=1, space="PSUM") as xpsum,
            tc.tile_pool(name="spsum", bufs=1, space="PSUM") as spsum,
            tc.tile_pool(name="hpsum", bufs=1, space="PSUM") as hpsum,
        ):
            # gather-critical loads first (gpsimd queue feeds the gathers)
            idx_sb = const.tile([128, NTOK // 128], mybir.dt.int32)
            nc.gpsimd.dma_start(out=idx_sb[:], in_=idx_d[:])
            iden_sb = const.tile([128, 128], bf16)
            nc.gpsimd.dma_start(out=iden_sb[:], in_=iden_d[:])
            # small conv weights on the sync HWDGE queue (ahead of the
            # per-chunk g2 shift DMAs, so those never sit behind big loads);
            # the big LSTM/head weights ride the scalar HWDGE queue, whose
            # engine is idle until the first relu
            convw8_sb = const.tile([128, NTAPS, 2, NFP], f8)
            nc.sync.dma_start(
                out=convw8_sb[:].rearrange("p a b c -> p (a b c)"),
                in_=convw8_d[:].rearrange("p a b c -> p (a b c)"),
            )
            convw2_sb = const.tile([128, 5 * NF], bf16)
            nc.sync.dma_start(out=convw2_sb[:], in_=convw2_d[:])
            convwS_sb = const.tile([128, 2 * NF], bf16)
            nc.sync.dma_start(out=convwS_sb[:], in_=convwS_d[:])
            convb_sb = const.tile([NF, 3], f32)
            nc.sync.dma_start(out=convb_sb[:], in_=convb_d[:])
            wih_sb = const.tile([NF + 1, 3, 2, G4], bf16)
            nc.scalar.dma_start(out=wih_sb[:], in_=wih_d[:])
            whh_sb = const.tile([128, 2, 2, 8, 128], f8)
            nc.scalar.dma_start(out=whh_sb[:], in_=whh_d[:])
            headw_sb = const.tile([128, 4, OUT], f8)
            nc.scalar.dma_start(out=headw_sb[:], in_=headw_d[:])
            headb_sb = const.tile([OUT, 1], f32)
            nc.scalar.dma_start(out=headb_sb[:], in_=headb_d[:])

            feats = [const.tile([128, S_CORE], bf16, tag=f"f{fs}", name=f"f{fs}") for fs in FSIZES]
            for fi in range(3):
                # constant-1 row 100 folds the (x16-scaled) gate bias into the
                # kk=0 xg matmul (wih row 100). Partition writes must start at
                # a 32-aligned base, so set rows 96:128; the conv reduce_max
                # overwrites rows 96:100 before anything reads them.
                nc.vector.memset(feats[fi][96:128, :], 1.0)
            # xgT: [128, (d, t, gt, b)] bf16 = x16-scaled input projections
            xgT = const.tile([128, 2, T, 8, B_CORE], bf16)
            # hseq: [128, slot, dir, ktile, b] fp8; slot 0 = h0 = 0
            hseq = const.tile([128, T + 1, 2, 2, B_CORE], f8)
            nc.vector.memset(hseq[:, 0], 0.0)
            c_prev = []
            for d in range(2):
                c0 = cstate.tile([128, 2 * B_CORE], f32, tag=f"c{d}", name=f"c0{d}")
                nc.vector.memset(c0[:], 0.0)
                c_prev.append(c0)
            # AMR per-partition accumulator scratch (result unused)
            amr_scratch = [
                const.tile([128, 1], f32, tag=f"amr{d}", name=f"amr{d}") for d in range(2)
            ]

            # ---------------- LSTM dir-step (independent chains) ----------------
            def lstm_dir_step(d, s):
                tt = s if d == 0 else T - 1 - s
                rslot = (s if d == 0 else (T + 1 - s)) if s > 0 else 0
                ps = spsum.tile([128, 32], f32, tag=f"gp{d}", name=f"gp{d}")
                # 16*whh @ h (fp8, FWL weight loads)
                for gt in range(8):
                    for kk in range(2):
                        nc.tensor.matmul(
                            ps[:, 4 * gt : 4 * gt + 4],
                            whh_sb[:, d, kk, gt, :],
                            hseq[:, rslot, d, kk, :],
                            start=(kk == 0),
                            stop=(kk == 1),
                        )
                gates = small.tile([128, 32], f32, tag=f"gates{d}", name=f"gates{d}")
                nc.vector.tensor_add(gates[:], ps[:], xgT[:, d, tt].rearrange("p g b -> p (g b)"))
                # cols 0:8=i, 8:16=f, 16:24=o, 24:32=sigma(2g); scale undoes x16
                sig = small.tile([128, 32], f32, tag=f"sig{d}", name=f"sig{d}")
                nc.scalar.activation(sig[:], gates[:], AF.Sigmoid, scale=1.0 / WS)
                # u = sig_i * (2*sig_2g - 1) = sig_i * tanh(g), one fused DVE op
                u = small.tile([128, 8], f32, tag=f"u{d}", name=f"u{d}")
                nc.vector.affine_mul_reduce(
                    u[:], amr_scratch[d][:], sig[:, 24:32], sig[:, 0:8], 2.0, -1.0
                )
                a = small.tile([128, 8], f32, tag=f"a{d}", name=f"a{d}")
                nc.vector.tensor_mul(a[:], sig[:, 8:16], c_prev[d][:])
                cn = cstate.tile([128, 2 * B_CORE], f32, tag=f"c{d}", name=f"c{d}")
                nc.vector.tensor_add(cn[:], a[:], u[:])
                c_prev[d] = cn
                thc = small.tile([128, 8], f32, tag=f"thc{d}", name=f"thc{d}")
                nc.scalar.activation(thc[:], cn[:], AF.Tanh)
                nc.vector.tensor_mul(hseq[:, tt + 1, d], sig[:, 16:24], thc[:])

            prog = {"fwd": 0, "rev": 0}
            done_chunks = set()

            def ready(dname):
                k = prog[dname]
                if k >= T:
                    return False
                ch = (k // CH_T) if dname == "fwd" else ((T - 1 - k) // CH_T)
                return ch in done_chunks

            def pump(budget):
                n = 0
                while n < budget:
                    progress = False
                    for dname, d in (("fwd", 0), ("rev", 1)):
                        if n >= budget:
                            break
                        if ready(dname):
                            lstm_dir_step(d, prog[dname])
                            prog[dname] += 1
                            n += 1
                            progress = True
                    if not progress:
                        break

            # ---------------- conv chunk ----------------
            def conv_chunk(sc):
                gA = gatA.tile([128, 2, CHTOK], f8, tag="gA", name="gA")
                g2 = gatB.tile([128, CHTOK], bf16, tag="g2", name="g2")

                def fetch(c):
                    gc = GCH * sc + c
                    gt = gtok.tile([128, EMB_PAD], bf16, tag="gt", name="gt")
                    nc.gpsimd.indirect_dma_start(
                        out=gt[:], out_offset=None, in_=emb_d[:],
                        in_offset=bass.IndirectOffsetOnAxis(
                            ap=idx_sb[:, gc : gc + 1], axis=0
                        ),
                    )
                    # three 128-col slabs via PE transpose into one PSUM bank;
                    # slab 2 rows 44:128 land zero (emb pad); g2 rows 64:128
                    # come from the per-block shift DMAs
                    tp = tpsum.tile([128, 3, 128], bf16, tag="tp", name="tp")
                    for e in range(3):
                        nc.tensor.transpose(
                            tp[:, e], gt[:, 128 * e : 128 * (e + 1)], iden_sb[:]
                        )
                    col = 128 * c
                    nc.vector.tensor_copy(
                        out=gA[:, :, col : col + 128], in_=tp[:, 0:2]
                    )
                    nc.vector.tensor_copy(
                        out=g2[0:64, col : col + 128], in_=tp[0:64, 2]
                    )

                def shift(j):
                    # tap-tail pairing for psum block j: rows 64:128 of g2 =
                    # rows 0:64 shifted by one token (rows 44:64 are zero ->
                    # rows 108:128 stay finite zeros). Block j's pair matmuls
                    # read shifted cols up to 384j+385, so each shift covers
                    # [384j+4, 384j+388).
                    lo = PS_COLS * j + 4 if j > 0 else 0
                    hi = min(PS_COLS * (j + 1) + 4, CHTOK - 1)
                    nc.sync.dma_start(
                        out=g2[64:128, lo:hi], in_=g2[0:64, lo + 1 : hi + 1]
                    )
                    if j == 3:
                        nc.vector.memset(g2[64:128, CHTOK - 1 : CHTOK], 0.0)

                fetched = 0
                for j in range(CHSENT // SENT_PER_PS):  # 4 psum tiles
                    # MM group j reads gA/g2 cols up to 384(j+1)+4, i.e.
                    # gather chunks up to 3j+3 — fetch just enough, so the
                    # first matmuls start after 4 gathers instead of 12
                    need = min(GCH, 3 * (j + 1) + 1)
                    while fetched < need:
                        fetch(fetched)
                        fetched += 1
                    shift(j)
                    base = PS_COLS * j
                    s0 = CHSENT * sc + SENT_PER_PS * j
                    for fi, fs in enumerate(FSIZES):
                        pt = (3 * j + fi) % 2  # alternate 2 banks for overlap
                        ps = cpsum.tile([NF, PS_COLS], f32, tag=f"ps{pt}", name=f"ps{pt}")
                        mms = []
                        # fp8 DoubleRow MMs over emb dims 0:256, one per tap
                        for k in range(fs):
                            ti = tap_of[fs] + k
                            mms.append(("dr", ti, k, 0))
                        for (pa, pb) in PAIRS_OF[fs]:
                            pi = PAIR_IDX[(fs, pa)]
                            mms.append(("pair", pi, pa, 1))
                        for k in SINGLES_OF[fs]:
                            si = SINGLE_IDX[(fs, k)]
                            mms.append(("single", si, k, 0))
                        for mm, (kind, wi, k, is_pair) in enumerate(mms):
                            n = min(PS_COLS, CHTOK - is_pair - base - k)
                            if kind == "dr":
                                nc.tensor.matmul(
                                    ps[:, 0:n],
                                    convw8_sb[:, wi, :, 0:NF],
                                    gA[:, :, base + k : base + k + n],
                                    start=(mm == 0),
                                    stop=(mm == len(mms) - 1),
                                    perf_mode=mybir.MatmulPerfMode.DoubleRow,
                                )
                            else:
                                wsb = convw2_sb if kind == "pair" else convwS_sb
                                nc.tensor.matmul(
                                    ps[:, 0:n],
                                    wsb[:, wi * NF : (wi + 1) * NF],
                                    g2[:, base + k : base + k + n],
                                    start=(mm == 0),
                                    stop=(mm == len(mms) - 1),
                                )
                        ps3 = ps[:].rearrange("p (s l) -> p s l", l=L)
                        nc.vector.reduce_max(
                            out=feats[fi][0:NF, s0 : s0 + SENT_PER_PS],
                            in_=ps3[:, :, 0 : L - fs + 1],
                            axis=mybir.AxisListType.X,
                        )
                    pump(1)
                for fi in range(3):
                    sl = slice(CHSENT * sc, CHSENT * (sc + 1))
                    nc.scalar.activation(
                        out=feats[fi][0:NF, sl], in_=feats[fi][0:NF, sl], func=AF.Relu,
                        bias=convb_sb[:, fi : fi + 1],
                    )

            def xg_chunk(sc):
                cols = slice(CHSENT * sc, CHSENT * (sc + 1))  # feats cols (t,b)
                for d in range(2):
                    ps = xpsum.tile([128, 8, CHSENT], f32, tag="xp", name="xp")
                    for gt in range(8):
                        for kk in range(3):
                            nc.tensor.matmul(
                                ps[:, gt],
                                wih_sb[:, kk, d, 128 * gt : 128 * (gt + 1)],
                                feats[kk][0 : NF + 1, cols],
                                start=(kk == 0),
                                stop=(kk == 2),
                            )
                    # psum (gt, t, b) -> xgT (t, gt, b), one strided bulk copy
                    nc.scalar.copy(
                        out=xgT[:, d, CH_T * sc : CH_T * (sc + 1), :, :],
                        in_=ps[:].rearrange("p g (t b) -> p t g b", b=B_CORE),
                    )
                    pump(1)

            for sc in CONV_ORDER:
                conv_chunk(sc)
                xg_chunk(sc)
                done_chunks.add(sc)
                pump(2)

            pump(10**9)
            assert prog["fwd"] == T and prog["rev"] == T

            # head: out.T[o, (b,t)] = sigmoid((headw*HS) @ h2 / HS + b)
            hp = hpsum.tile([OUT, S_CORE], f32)
            for qd in range(4):
                d, kk = qd // 2, qd % 2
                rhs = hseq[:, 1 : T + 1, d, kk, :].rearrange("p t b -> p b t")
                nc.tensor.matmul(
                    hp[:], headw_sb[:, qd, :], rhs, start=(qd == 0), stop=(qd == 3)
                )
            out_sb = small.tile([OUT, S_CORE], f32, tag="outsb", name="outsb")
            nc.scalar.activation(
                out_sb[:], hp[:], AF.Sigmoid, bias=headb_sb[:], scale=1.0 / HS
            )
            nc.gpsimd.dma_start(out=out_d[:], in_=out_sb[:])

    nc.compile()
    return nc


def get_program():
    global _PROG
    if _PROG is None:
        _PROG = build_program()
    return _PROG


# ------------- host-side data prep (reshape/transpose/pad/cast only) -------------

def prep_shared(inputs):
    emb = np.zeros((VOCAB, EMB_PAD), np.float32)
    emb[:, :EMB] = inputs["emb"]
    emb_p = emb.astype(BF16)

    # fp8 DoubleRow weights for emb dims 0:256: [p, tap, j, f] = W[f, 128j+p, k]
    convw8 = np.zeros((128, NTAPS, 2, NFP), np.float32)
    for fs in FSIZES:
        w = np.asarray(inputs[f"conv_w{fs}"], np.float32)  # [NF, EMB, fs]
        for k in range(fs):
            ti = tap_of_host(fs) + k
            blk = w[:, 0:256, k].T.reshape(2, 128, NF).transpose(1, 0, 2)
            convw8[:, ti, :, 0:NF] = blk
    convw8 = convw8.astype(F8)

    convb = np.stack(
        [np.asarray(inputs[f"conv_b{fs}"], np.float32) for fs in FSIZES], axis=1
    )

    # bf16 pair weights for emb dims 256:300 (rows 0:44 tap a, 64:108 tap a+1)
    convw2 = np.zeros((128, 5 * NF), np.float32)
    for i, (fs, ka) in enumerate(PAIR_LIST):
        w = np.asarray(inputs[f"conv_w{fs}"], np.float32)
        convw2[0:44, i * NF : (i + 1) * NF] = w[:, 256:300, ka].T
        convw2[64:108, i * NF : (i + 1) * NF] = w[:, 256:300, ka + 1].T

    # bf16 single-tap weights for emb dims 256:300 (rows 0:44)
    convwS = np.zeros((128, 2 * NF), np.float32)
    for i, (fs, k) in enumerate(SINGLE_LIST):
        w = np.asarray(inputs[f"conv_w{fs}"], np.float32)
        convwS[0:44, i * NF : (i + 1) * NF] = w[:, 256:300, k].T

    perm = np.concatenate(
        [np.arange(0, 256), np.arange(256, 512), np.arange(768, 1024), np.arange(512, 768)]
    )  # i,f,g,o -> i,f,o,g

    wih_h = np.zeros((NF + 1, 3, 2, G4), np.float32)
    whh_h = np.zeros((128, 2, 2, 8, 128), np.float32)
    for d, tag in ((0, "f"), (1, "r")):
        wih = np.asarray(inputs[f"w_ih_{tag}"], np.float32)[perm]
        whh = np.asarray(inputs[f"w_hh_{tag}"], np.float32)[perm]
        bsum = (
            np.asarray(inputs[f"b_ih_{tag}"], np.float32)
            + np.asarray(inputs[f"b_hh_{tag}"], np.float32)
        )[perm].copy()
        # double the g-gate rows (768:1024 after perm) so sigmoid gives sigma(2g)
        wih[768:1024] *= 2.0
        whh[768:1024] *= 2.0
        bsum[768:1024] *= 2.0
        # x16 so whh lands in fp8's normal range; sigmoid applies 1/16
        wih *= WS
        whh *= WS
        bsum *= WS
        for kk in range(3):
            wih_h[0:NF, kk, d, :] = wih[:, kk * NF : (kk + 1) * NF].T
        wih_h[NF, 0, d, :] = bsum  # bias row rides the kk=0 matmul
        whh_h[:, d] = whh.reshape(8, 128, 2, 128).transpose(3, 2, 0, 1)

    headw = np.asarray(inputs["head_w"], np.float32) * HS
    headw_h = headw.T.reshape(4, 128, OUT).transpose(1, 0, 2).astype(F8)
    headb_h = np.asarray(inputs["head_b"], np.float32).reshape(OUT, 1)

    return {
        "emb_p": emb_p,
        "convw8": np.ascontiguousarray(convw8),
        "convw2": np.ascontiguousarray(convw2.astype(BF16)),
        "convwS": np.ascontiguousarray(convwS.astype(BF16)),
        "convb": np.ascontiguousarray(convb),
        "wih": wih_h.astype(BF16),
        "whh": np.ascontiguousarray(whh_h.astype(F8)),
        "headw": np.ascontiguousarray(headw_h),
        "headb": headb_h,
        "iden": np.eye(128, dtype=BF16),
    }


def tap_of_host(fs):
    return {3: 0, 4: 3, 5: 7}[fs]


def prep_core_idx(dialogue, core):
    """(t, b)-ordered token stream; token c*128+p at [p, c]."""
    dia = np.asarray(dialogue[B_CORE * core : B_CORE * (core + 1)], np.int32)
    ids = dia.transpose(1, 0, 2).reshape(-1)  # (t, b, l)
    return np.ascontiguousarray(ids.reshape(NTOK // 128, 128).T)


def kernel(**inputs):
    from concourse.bass_utils import run_bass_kernel_spmd

    nc = get_program()
    shared = prep_shared(inputs)
    dialogue = np.asarray(inputs["dialogue"])
    in_maps = []
    for core in range(NCORES):
        m = dict(shared)
        m["idx_w"] = prep_core_idx(dialogue, core)
        in_maps.append(m)
    res = run_bass_kernel_spmd(nc, in_maps, list(range(NCORES)))
    out = np.zeros((B, T, OUT), np.float32)
    for core in range(NCORES):
        o = res.results[core]["out_t"]  # [32, 256] col = b*64 + t
        out[B_CORE * core : B_CORE * (core + 1)] = o.reshape(OUT, B_CORE, T).transpose(
            1, 2, 0
        )
    return out
